# revision 20
# baseline (speedup 1.0000x reference)
"""Trainium2 Bass kernel for nn_Decoder_40338332844507.

Computes logits = einsum('btc,wpc->bptw', q, W) + b.T[None,:,None,:]
with q [32, 2048, 256] f32, W [49, 32, 256] f32, b [49, 32] f32,
output [32, 32, 2048, 49] f32.

Strategy: data-parallel over batch across 8 NeuronCores (4 batches per
core). Per core, for each 128-token tile the TensorEngine computes
out[t, (p,w)] = qT_tile.T @ Wr in bf16 (fp32 matmuls lower to slow
LOW_HIGH pairs and keep the PE clock throttled). The DVE eviction
PSUM->SBUF fuses the f32 bias add. Token tiles are strided
(t = tp*TL + tl, partition dim = tp) so that each output store covers
a fully contiguous DRAM region with 16*49*4-byte descriptor runs,
which sustains ~360-400 GB/s on the store stream (vs ~280 GB/s for
128-token-contiguous tiling). The first batch is split into smaller
fills so stores start as early as possible; the store stream is the
critical path (~51 MB/core at the HBM ceiling).
"""

import json
import sys
import numpy as np
from contextlib import ExitStack

if "/opt/trn_rl_repo" not in sys.path:
    sys.path.insert(0, "/opt/trn_rl_repo")

import concourse.bass as bass
import concourse.tile as tile
from concourse import mybir
from concourse.bass_utils import run_bass_kernel_spmd

B, T, C = 32, 2048, 256
P, WW = 32, 49
N = P * WW  # 1568
N_CORES = 8
B_LOC = B // N_CORES  # 4 batches per core
TL = 16  # token interleave: t = tp*16 + tl -> store runs of 16*49*4 B


def _patch_split_sync_waits():
    """The walrus build on this image accepts at most ONE sync-wait per
    instruction ("Too many sync wait commands" otherwise). Tile emits
    instructions with several waits. Post-process the serialized BIR:
    hoist all but the last wait of each instruction onto 1-wait NoOps
    inserted immediately before it on the same engine (engines execute
    their instruction stream in order, so the semantics are identical)."""
    if getattr(bass.Bass, "_split_waits_patched", False):
        return
    orig = bass.Bass.to_json_bytes

    def to_json_bytes(self):
        m = json.loads(orig(self))
        ctr = 0
        for f in m.get("functions", []):
            for bb in f.get("blocks", []):
                out = []
                for inst in bb.get("instructions", []):
                    si = inst.get("sync_info")
                    if si:
                        waits = si.get("on_wait") or []
                        if len(waits) > 1:
                            for wt in waits[:-1]:
                                ctr += 1
                                nop = {
                                    "engine": inst["engine"],
                                    "ins": [],
                                    "outs": [],
                                    "name": f"I-npw{ctr}",
                                    "opcode": "NoOp",
                                    "sync_info": {"on_wait": [wt], "on_update": []},
                                }
                                if inst.get("debug") is not None:
                                    nop["debug"] = inst["debug"]
                                out.append(nop)
                            si["on_wait"] = waits[-1:]
                    out.append(inst)
                bb["instructions"] = out
        return json.dumps(m).encode()

    bass.Bass.to_json_bytes = to_json_bytes
    bass.Bass._split_waits_patched = True


def build_bass():
    _patch_split_sync_waits()
    nc = bass.Bass("TRN2", target_bir_lowering=False, debug=False)
    f32 = mybir.dt.float32
    bf16 = mybir.dt.bfloat16

    qt = nc.dram_tensor("qt", [B_LOC, C, T], bf16, kind="ExternalInput")
    wr = nc.dram_tensor("wr", [C, N], bf16, kind="ExternalInput")
    bf_full = nc.dram_tensor("bf_full", [128, N], f32, kind="ExternalInput")
    o = nc.dram_tensor("o", [B_LOC, P, T, WW], f32, kind="ExternalOutput")

    with tile.TileContext(nc) as tc:
        with ExitStack() as ctx:
            consts = ctx.enter_context(tc.tile_pool(name="consts", bufs=1))
            qpool = ctx.enter_context(tc.tile_pool(name="qpool", bufs=2))
            opool = ctx.enter_context(tc.tile_pool(name="opool", bufs=3))
            psum = ctx.enter_context(tc.tile_pool(name="psum", bufs=4, space="PSUM"))

            wr_sb = [
                consts.tile([128, N], bf16, tag=f"wr{k}", name=f"wr{k}")
                for k in range(2)
            ]
            nc.sync.dma_start(wr_sb[0][:], wr.ap()[0:128, :])
            nc.scalar.dma_start(wr_sb[1][:], wr.ap()[128:256, :])
            bias_sb = consts.tile([128, N], f32, tag="bias", name="bias_sb")
            nc.gpsimd.dma_start(bias_sb[:], bf_full.ap()[:, :])
            bias_v = bias_sb[:].rearrange("t (p w) -> t p w", w=WW)

            def fill_and_store(b, q_v, p0, np_, store_engines, name):
                """Compute o[b, p0:p0+np_] into one [tp=128, np_, TL*WW] tile
                and store it as len(store_engines) contiguous DMAs."""
                oh = opool.tile([128, np_, TL * WW], f32, tag="obig", name=name)
                for tl in range(TL):
                    pt = psum.tile([128, 1024], f32, tag="pt", name=f"pt_{name}_{tl}")
                    nw = np_ * WW
                    for n0 in range(0, nw, 512):
                        n1 = min(n0 + 512, nw)
                        nc.tensor.matmul(
                            pt[:, n0:n1],
                            q_v[0][:, tl, :],
                            wr_sb[0][:, p0 * WW + n0 : p0 * WW + n1],
                            start=True,
                            stop=False,
                        )
                        nc.tensor.matmul(
                            pt[:, n0:n1],
                            q_v[1][:, tl, :],
                            wr_sb[1][:, p0 * WW + n0 : p0 * WW + n1],
                            start=False,
                            stop=True,
                        )
                    pv = pt[:, :nw].rearrange("t (p w) -> t p w", w=WW)
                    nc.vector.tensor_add(
                        oh[:, :, bass.ds(tl * WW, WW)],
                        pv[:],
                        bias_v[:, p0 : p0 + np_, :],
                    )
                ns = len(store_engines)
                ps = np_ // ns
                for s, eng in enumerate(store_engines):
                    dst = (
                        o.ap()[b, p0 + ps * s : p0 + ps * (s + 1), :, :]
                        .rearrange("p (t l) w -> t p (l w)", l=TL)
                    )
                    eng.dma_start(dst, oh[:, ps * s : ps * (s + 1), :])

            for b in range(B_LOC):
                # load q[b] transposed: two [128(c), 2048(t)] bf16 tiles
                q_sb = [
                    qpool.tile([128, T], bf16, tag=f"q{k}", name=f"q{k}_{b}")
                    for k in range(2)
                ]
                nc.gpsimd.dma_start(q_sb[0][:], qt.ap()[b, 0:128, :])
                nc.gpsimd.dma_start(q_sb[1][:], qt.ap()[b, 128:256, :])
                # t split as (tp, tl); lhsT tiles are [c, tp] (stride TL)
                q_v = [
                    q_sb[k][:].rearrange("c (p l) -> c l p", l=TL) for k in range(2)
                ]

                if b == 0:
                    # prime the store pipeline: quarter-size fills keep the
                    # store supply continuous from t~9us until steady state
                    engs = [nc.sync, nc.scalar, nc.sync, nc.scalar]
                    for qd in range(4):
                        fill_and_store(b, q_v, 8 * qd, 8, [engs[qd]], f"ohq{qd}")
                else:
                    for h in range(2):
                        fill_and_store(
                            b, q_v, 16 * h, 16, [nc.sync, nc.scalar], f"oh{b}{h}"
                        )
    return nc


_NC_CACHE = None


def _get_nc():
    global _NC_CACHE
    if _NC_CACHE is None:
        _NC_CACHE = build_bass()
    return _NC_CACHE


def kernel(q, W, b):
    import ml_dtypes

    bf = ml_dtypes.bfloat16
    Wt = np.asarray(W, dtype=np.float32)
    bias = np.asarray(b, dtype=np.float32)
    q = np.asarray(q, dtype=np.float32)

    # host-side layout prep (weight packing + activation transpose + bf16 cast)
    qt = np.ascontiguousarray(q.transpose(0, 2, 1).astype(bf))  # [B, C, T]
    wr = np.ascontiguousarray(Wt.transpose(2, 1, 0).reshape(C, N).astype(bf))
    bf_full = np.ascontiguousarray(
        np.broadcast_to(bias.T.reshape(1, N), (128, N)).astype(np.float32)
    )

    nc = _get_nc()
    in_maps = [
        {
            "qt": qt[c * B_LOC : (c + 1) * B_LOC],
            "wr": wr,
            "bf_full": bf_full,
        }
        for c in range(N_CORES)
    ]
    res = run_bass_kernel_spmd(nc, in_maps, core_ids=list(range(N_CORES)))
    out = np.concatenate([res.results[c]["o"] for c in range(N_CORES)], axis=0)
    return out
